# revision 1
# baseline (speedup 1.0000x reference)
"""ConvCapsuleLayer Trainium2 kernel (8-core SPMD, data-parallel over batch).

Reference computation (see problem):
  x [16,32,32,8,16] -> transpose/merge -> conv5x5 SAME (16->256) on 128 images
  -> votes [B=16,I=8,32,32,O=16,D=16] -> 3 dynamic-routing iterations
  -> activation [16,32,32,16,16].

Sharding: conv image k = 8*b' + i' (b' = routing batch, i' = input capsule).
Core c owns routing batches b' in {2c, 2c+1} = conv images k in [16c,16c+16),
which is exactly x[:, :, :, c, :] (b_ref = k%16, i_ref = k//16 = c).
Everything (conv + routing) is core-local; no collectives.

Per-core program:
  - conv as PE matmuls: stationary = 5-row-shifted input copies XS[(ky,ci)=80,
    pixel window 128 = 4 y-rows x 32 x], moving = W[(ky,ci), 256 co], fp32r,
    accumulated over the 5 kx taps into PSUM -> votes land directly in
    pixel-partition layout [128 pixels, (i, o, d)].
  - routing on Vector engine with a custom fused DVE op DOT_SCAN_ANT
    (prefix-sum of Src0*Src1) doing multiply+segmented-reduce in one pass
    (segment sums recovered by differencing the prefix at segment ends);
    exp/sqrt on Scalar engine; exact DVE reciprocal for divisions; fp32
    everywhere.
"""

import os
import numpy as np

import concourse.bass as bass
import concourse.bacc as bacc
import concourse.mybir as mybir
import concourse.tile as tile
from concourse import bass_utils

# ----------------------------------------------------------------------------
# Problem constants (hardcoded; kernel.py must be self-contained)
B_FULL, H, Wd, I, DIN = 16, 32, 32, 8, 16
O, D = 16, 16
CO = O * D            # 256 conv output channels
KK = 5                # kernel spatial size
KCI = KK * DIN        # 80 = contraction (ky, ci)
N_CORES = 8
B_LOC = 2             # routing batches per core
N_IMG = 16            # conv images per core
ROUTINGS = 3

# Routing seg partitioning: seg = (b, tg); each seg covers L y-tiles (4 rows each)
L = 2                 # y-tiles per routing seg
N_TG = 8 // L         # y-tile groups per b
SEG_FREE = I * L * CO   # 4096 votes elems per partition per seg
M_STREAM = L * CO       # 512  merged (dt, od)
J_STREAM = I * L        # 16   merged (i, dt)

F32 = mybir.dt.float32
F32R = mybir.dt.float32r
AX = mybir.AxisListType
ALU = mybir.AluOpType
ACTF = mybir.ActivationFunctionType

USE_SCAN = bool(int(os.environ.get("USE_SCAN", "1")))  # fused DOT_SCAN vs stock

# ----------------------------------------------------------------------------
# Custom DVE op: prefix-sum of element product, out[p,k] = sum_{t<=k} in0*in1
_DOT_SCAN = None


def _get_dot_scan():
    global _DOT_SCAN
    if _DOT_SCAN is not None:
        return _DOT_SCAN
    import concourse.dve_ops as dvo
    from concourse.dve_spec import Spec, Src0, Src1, AluOp, lower, scan
    from concourse.dve_uop import DveOpSpec

    name = "DOT_SCAN_ANT"

    def _ref(in0, in1, s0, s1, imm2):
        p = in0.shape[0]
        a = np.asarray(in0, np.float32).reshape(p, -1)
        b = np.asarray(in1, np.float32).reshape(p, -1)
        prod = (a * b).astype(np.float32)
        return np.cumsum(prod, axis=1, dtype=np.float32)

    spec = Spec(body=scan(AluOp.ADD, Src0 * Src1), reference=_ref)
    if name not in dvo._SUB_OPCODE_FOR_NAME:
        row = max(dvo._SUB_OPCODE_FOR_NAME.values()) + 1
        assert row < 0x20
        dvo._SUB_OPCODE_FOR_NAME[name] = row
    row = dvo._SUB_OPCODE_FOR_NAME[name]
    shas = {}
    for ver in ("v3", "v4"):
        try:
            uops = lower(spec, ver=ver)
            shas[ver] = DveOpSpec(name=name, opcode=row, uops=uops, rd1_en=True).sha(ver)
        except Exception:
            pass
    op = dvo.DveOp(name, spec, subdim=False, uops_sha=shas)
    if not any(o.name == name for o in dvo.OPS):
        dvo.OPS.append(op)
    dvo.CUSTOM_DVE_SPECS[name] = spec
    _DOT_SCAN = op
    return op


# ----------------------------------------------------------------------------
def _fv(t, base_off_elems, dims):
    """Free-dim view of an SBUF/PSUM tile AP: keep its partition dim, replace
    free dims with explicit [step, count] pairs at an element offset."""
    return bass.AP(tensor=t.tensor, offset=t.offset + base_off_elems,
                   ap=[t.ap[0]] + [list(d) for d in dims])


def _pv(t, base_off_elems, part_dim, dims):
    """View with explicit partition dim too (for partition sub-ranges)."""
    return bass.AP(tensor=t.tensor, offset=t.offset + base_off_elems,
                   ap=[list(part_dim)] + [list(d) for d in dims])


def build_program():
    """Build the (SPMD-identical) single-core Bass program."""
    if USE_SCAN:
        dot_scan = _get_dot_scan()
    nc = bacc.Bacc("TRN2", target_bir_lowering=False, debug=False)

    xs_d = nc.dram_tensor("xs", [KCI, N_IMG, Wd + 4, H], F32R, kind="ExternalInput")
    w_d = nc.dram_tensor("w", [KCI, KK * CO], F32R, kind="ExternalInput")
    b_d = nc.dram_tensor("b", [1, CO], F32, kind="ExternalInput")
    out_d = nc.dram_tensor("out", [B_LOC, H, Wd, CO], F32, kind="ExternalOutput")

    with tile.TileContext(nc) as tc:
        with (
            tc.tile_pool(name="persist", bufs=1) as persist,
            tc.tile_pool(name="votes", bufs=2) as votes_pool,
            tc.tile_pool(name="small2", bufs=2) as small2,
            tc.tile_pool(name="psum", bufs=2, space="PSUM") as psum_pool,
        ):
            # ---- constants / inputs in SBUF
            xs = persist.tile([KCI, N_IMG, Wd + 4, H], F32R, tag="xs")
            for n in range(N_IMG):
                nc.sync.dma_start(out=xs[:, n, :, :], in_=xs_d.ap()[:, n, :, :])
            wsb = persist.tile([KCI, KK * CO], F32R, tag="wsb")
            nc.sync.dma_start(out=wsb[:], in_=w_d.ap())
            bias = persist.tile([128, CO], F32, tag="bias")
            b_ap = b_d.ap()
            nc.sync.dma_start(
                out=bias[:],
                in_=bass.AP(tensor=b_ap.tensor, offset=0, ap=[[0, 128], [1, CO]]),
            )
            ones = persist.tile([128, 1], F32, tag="ones")
            nc.vector.memset(ones[:], 1.0)

            # persistent scratch (DVE-only consumers -> single buffer is fine)
            S = persist.tile([128, 1 + SEG_FREE], F32, tag="S")       # big scan
            S2 = persist.tile([128, 1 + M_STREAM], F32, tag="S2")     # sq scan
            nc.vector.memset(S[:, 0:1], 0.0)
            nc.vector.memset(S2[:, 0:1], 0.0)
            route_d = persist.tile([128, SEG_FREE], F32, tag="route_d")
            preact = persist.tile([128, M_STREAM], F32, tag="preact")
            delta = persist.tile([128, J_STREAM * O], F32, tag="delta")
            den = persist.tile([128, L * O], F32, tag="den")
            rden = persist.tile([128, L * O], F32, tag="rden")
            sqn = persist.tile([128, L * O], F32, tag="sqn")
            tsc = persist.tile([128, L * O], F32, tag="tsc")
            sden = persist.tile([128, J_STREAM], F32, tag="sden")
            srden = persist.tile([128, J_STREAM], F32, tag="srden")

            for b in range(B_LOC):
                for tg in range(N_TG):
                    # ---- conv for this seg --------------------------------
                    votes = votes_pool.tile([128, I, L, CO], F32, tag="votes")
                    for dt in range(L):
                        t = tg * L + dt
                        ps = psum_pool.tile([128, I, CO], F32, tag="ps")
                        for i in range(I):
                            n = b * I + i
                            for kx in range(KK):
                                # stationary = 4 x-cols x 32 y, contiguous 128
                                lhs = _fv(xs,
                                          (n * (Wd + 4) + 4 * t + kx) * H,
                                          [[1, 128]])
                                rhs = _fv(wsb, kx * CO, [[1, CO]])
                                nc.tensor.matmul(
                                    ps[:, i, :],
                                    lhsT=lhs,
                                    rhs=rhs,
                                    start=(kx == 0),
                                    stop=(kx == KK - 1),
                                )
                        # evacuate psum -> votes[:, :, dt, :]
                        nc.scalar.copy(
                            out=_fv(votes, dt * CO, [[L * CO, I], [1, CO]]),
                            in_=ps[:, :, :],
                        )

                    # ---- routing for this seg -----------------------------
                    logits = small2.tile([128, J_STREAM * O], F32, tag="logits")
                    exps = small2.tile([128, J_STREAM * O], F32, tag="exps")
                    route = small2.tile([128, J_STREAM * O], F32, tag="route")
                    n2 = small2.tile([128, L * O], F32, tag="n2")
                    act = small2.tile([128, M_STREAM], F32, tag="act")

                    # views reused across iterations
                    # votes as stream (m=(dt,od), i): [p][m:512 str1][i:8 str512]
                    v_mi = _fv(votes, 0, [[1, M_STREAM], [M_STREAM, I]])
                    # votes as stream (j=(i,dt), od): [p][j:16 str256][od:256 str1]
                    v_jod = _fv(votes, 0, [[CO, J_STREAM], [1, CO]])

                    for it in range(ROUTINGS):
                        if it > 0:
                            # softmax over o: exps, denom, recip, route
                            nc.scalar.activation(out=exps[:], in_=logits[:],
                                                 func=ACTF.Exp)
                            nc.vector.tensor_reduce(
                                out=sden[:], op=ALU.add, axis=AX.X,
                                in_=_fv(exps, 0, [[O, J_STREAM], [1, O]]))
                            nc.vector.reciprocal(out=srden[:], in_=sden[:])
                            nc.vector.tensor_mul(
                                route[:], exps[:],
                                _fv(srden, 0, [[1, J_STREAM], [0, O]]))
                            # expand route[(i,dt,o)] -> route_d[(dt,od),i]
                            # out element (dt,o,d,i) at dt*2048 + o*128 + d*8 + i
                            nc.scalar.activation(
                                out=_fv(route_d, 0,
                                        [[O * CO // 2, L], [CO // 2, O],
                                         [I, D], [1, I]]),
                                in_=_fv(route, 0, [[O, L], [1, O], [0, D], [O * L, I]]),
                                func=ACTF.Copy)

                        # preact_raw[m] = sum_i route*votes  (fused scan + diff)
                        if USE_SCAN:
                            nc.vector._custom_dve(
                                dot_scan, out=S[:, 1:], in0=v_mi,
                                in1=(_fv(ones, 0, [[0, SEG_FREE]]) if it == 0
                                     else route_d[:]))
                            nc.vector.tensor_sub(
                                preact[:],
                                _fv(S, 1 + (I - 1), [[I, M_STREAM]]),
                                _fv(S, 0, [[I, M_STREAM]]))
                        else:
                            if it == 0:
                                nc.vector.tensor_reduce(
                                    out=preact[:], op=ALU.add, axis=AX.X, in_=v_mi)
                            else:
                                nc.vector.tensor_mul(
                                    _fv(S, 1, [[1, M_STREAM], [M_STREAM, I]]),
                                    v_mi,
                                    _fv(route_d, 0, [[I, M_STREAM], [1, I]]))
                                nc.vector.tensor_reduce(
                                    out=preact[:], op=ALU.add, axis=AX.X,
                                    in_=_fv(S, 1, [[1, M_STREAM], [M_STREAM, I]]))
                        # preact = preact_raw*scale + bias
                        nc.vector.scalar_tensor_tensor(
                            out=preact[:], in0=preact[:],
                            scalar=(1.0 / O) if it == 0 else 1.0,
                            in1=_fv(bias, 0, [[0, L], [1, CO]]),
                            op0=ALU.mult, op1=ALU.add)

                        # squash: n2 = sum_d preact^2 (scan+diff), t = sqrt/(1+n2)
                        if USE_SCAN:
                            nc.vector._custom_dve(
                                dot_scan, out=S2[:, 1:], in0=preact[:],
                                in1=preact[:])
                            nc.vector.tensor_sub(
                                n2[:],
                                _fv(S2, 1 + (D - 1), [[D, L * O]]),
                                _fv(S2, 0, [[D, L * O]]))
                        else:
                            nc.vector.tensor_mul(S2[:, 1:], preact[:], preact[:])
                            nc.vector.tensor_reduce(
                                out=n2[:], op=ALU.add, axis=AX.X,
                                in_=_fv(S2, 1, [[D, L * O], [1, D]]))
                        nc.vector.tensor_scalar_add(den[:], n2[:], 1.0)
                        nc.vector.reciprocal(out=rden[:], in_=den[:])
                        nc.scalar.activation(out=sqn[:], in_=n2[:], func=ACTF.Sqrt)
                        nc.vector.tensor_mul(tsc[:], sqn[:], rden[:])
                        nc.vector.tensor_mul(
                            act[:], preact[:],
                            _fv(tsc, 0, [[1, L * O], [0, D]]))

                        if it < ROUTINGS - 1:
                            # agreement: delta[(i,dt,o)] = sum_d votes*act
                            dtarget = logits if it == 0 else delta
                            if USE_SCAN:
                                nc.vector._custom_dve(
                                    dot_scan, out=S[:, 1:], in0=v_jod,
                                    in1=_fv(act, 0, [[0, I], [1, M_STREAM]]))
                                nc.vector.tensor_sub(
                                    dtarget[:],
                                    _fv(S, 1 + (D - 1), [[D, J_STREAM * O]]),
                                    _fv(S, 0, [[D, J_STREAM * O]]))
                            else:
                                nc.vector.tensor_mul(
                                    _fv(S, 1, [[1, SEG_FREE]]),
                                    v_jod,
                                    _fv(act, 0, [[0, I], [1, M_STREAM]]))
                                nc.vector.tensor_reduce(
                                    out=dtarget[:], op=ALU.add, axis=AX.X,
                                    in_=_fv(S, 1, [[D, J_STREAM * O], [1, D]]))
                            if it > 0:
                                nc.vector.tensor_add(logits[:], logits[:], delta[:])

                    # ---- write act back to HBM ----------------------------
                    # act[p=(xx,y), (dt, od)] -> out[b, y, 4*(tg*L+dt)+xx, od]
                    for xx in range(4):
                        dst = bass.AP(
                            tensor=out_d.ap().tensor,
                            offset=(b * H * Wd + 4 * (tg * L) + xx) * CO,
                            ap=[[Wd * CO, 32], [4 * CO, L], [1, CO]],
                        )
                        nc.sync.dma_start(
                            out=dst,
                            in_=act[32 * xx:32 * xx + 32, :].rearrange(
                                "p (l c) -> p l c", l=L))

    if not nc.is_finalized():
        nc.finalize()
    return nc


# ----------------------------------------------------------------------------
def _host_prep(x, W, b):
    """Build per-core input arrays."""
    x = np.asarray(x, np.float32)
    W = np.asarray(W, np.float32)
    b = np.asarray(b, np.float32)
    w2 = np.ascontiguousarray(W.transpose(0, 2, 1, 3).reshape(KCI, KK * CO))
    bvec = np.ascontiguousarray(b.reshape(1, CO))
    in_maps = []
    for c in range(N_CORES):
        xc = x[:, :, :, c, :]  # [16, 32, 32, 16]
        XS = np.zeros((KCI, N_IMG, H, Wd + 4), np.float32)
        for ky in range(KK):
            ylo = max(0, ky - 2)
            yhi = min(H, H + ky - 2)
            dlo, dhi = ylo - (ky - 2), yhi - (ky - 2)
            XS[16 * ky:16 * ky + 16, :, dlo:dhi, 2:2 + Wd] = \
                xc[:, ylo:yhi, :, :].transpose(3, 0, 1, 2)
        XS = XS.transpose(0, 1, 3, 2)  # -> [KCI, N_IMG, Wd+4, H]
        in_maps.append({"xs": np.ascontiguousarray(XS), "w": w2, "b": bvec})
    return in_maps


_NC_CACHE = None


def kernel(x, W, b):
    global _NC_CACHE
    if _NC_CACHE is None:
        _NC_CACHE = build_program()
    nc = _NC_CACHE
    in_maps = _host_prep(x, W, b)
    res = bass_utils.run_bass_kernel_spmd(
        nc, in_maps, core_ids=list(range(N_CORES)),
        trace=bool(int(os.environ.get("KERNEL_TRACE", "0"))),
    )
    out = np.empty((B_FULL, H, Wd, O, D), np.float32)
    for c in range(N_CORES):
        out[2 * c:2 * c + 2] = res.results[c]["out"].reshape(B_LOC, H, Wd, O, D)
    kernel.last_results = res
    return out



# revision 10
# speedup vs baseline: 4.5267x; 4.5267x over previous
"""ConvCapsuleLayer Trainium2 kernel (8-core SPMD, capsule-parallel).

Reference computation:
  x [16,32,32,8,16] -> transpose/merge -> conv5x5 SAME (16->256) on 128 images
  -> votes [B=16,I=8,32,32,O=16,D=16] -> 3 dynamic-routing iterations
  -> activation [16,32,32,16,16].

Sharding: conv image k = 8*b' + i' (b' = routing batch, i' = input capsule).
Core c owns routing batches b' in {2c, 2c+1} = conv images k in [16c,16c+16),
which is exactly x[:, :, :, c, :]. Everything is core-local; no collectives.

Wall-clock here is dominated by the host<->device tunnel (~40-50 MB/s) and a
~0.1 s dispatch latency, so the kernel is built to minimize bytes moved per
call:
  - x ships un-replicated as fp16 [ci,n,x,y] (4.2 MB total); the 5x
    ky-replicated conv layout is built on-device with 5 strided DMAs.
  - W ships fp16 (1.6 MB incl. the 8-core tile), b fp32 (tiny).
  - the output returns as fp16 (8.4 MB instead of 16.8).
  - the output device buffer is donated from the previous call instead of
    uploading fresh zeros every call.
  - the jit(shard_map(...)) executable is built once and cached, instead of
    being rebuilt (and re-traced) per call inside run_bass_kernel_spmd.

Per-core program:
  - conv as PE matmuls (fp16 in, fp32 PSUM): stationary = 5-row-shifted input
    copies XS[(ky,ci)=80, pixel window 128 = 4 x-cols x 32 y], moving =
    W[(ky,ci), 256 co], accumulated over the 5 kx taps into PSUM -> votes land
    directly in pixel-partition layout [128 pixels, (i, o, d)].
  - routing on Vector engine in fp32 with a custom fused DVE op DOT_SCAN_ANT
    (prefix-sum of Src0*Src1) doing multiply+segmented-reduce in one pass;
    exp/sqrt on Scalar engine; exact DVE reciprocal for divisions.
"""

import os
import numpy as np

import jax
from jax.sharding import Mesh, PartitionSpec
from jax.experimental.shard_map import shard_map

import concourse.bass as bass
import concourse.bacc as bacc
import concourse.mybir as mybir
import concourse.tile as tile

# ----------------------------------------------------------------------------
# Problem constants (hardcoded; kernel.py must be self-contained)
B_FULL, H, Wd, I, DIN = 16, 32, 32, 8, 16
O, D = 16, 16
CO = O * D            # 256 conv output channels
KK = 5                # kernel spatial size
KCI = KK * DIN        # 80 = contraction (ky, ci)
XP = Wd + 4           # x axis padded by 2 on each side
N_CORES = 8
B_LOC = 2             # routing batches per core
N_IMG = 16            # conv images per core
ROUTINGS = 3

# Routing seg partitioning: seg = (b, tg); each seg covers L x-tiles (4 cols)
L = 2                 # x-tiles per routing seg
N_TG = 8 // L         # x-tile groups per b
SEG_FREE = I * L * CO   # 4096 votes elems per partition per seg
M_STREAM = L * CO       # 512  merged (dt, od)
J_STREAM = I * L        # 16   merged (i, dt)

F32 = mybir.dt.float32
F16 = mybir.dt.float16
AX = mybir.AxisListType
ALU = mybir.AluOpType
ACTF = mybir.ActivationFunctionType

USE_SCAN = bool(int(os.environ.get("USE_SCAN", "1")))  # fused DOT_SCAN vs stock

# ----------------------------------------------------------------------------
# Custom DVE op: prefix-sum of element product, out[p,k] = sum_{t<=k} in0*in1
_DOT_SCAN = None


def _get_dot_scan():
    global _DOT_SCAN
    if _DOT_SCAN is not None:
        return _DOT_SCAN
    import concourse.dve_ops as dvo
    from concourse.dve_spec import Spec, Src0, Src1, AluOp, lower, scan
    from concourse.dve_uop import DveOpSpec

    name = "DOT_SCAN_ANT"

    def _ref(in0, in1, s0, s1, imm2):
        p = in0.shape[0]
        a = np.asarray(in0, np.float32).reshape(p, -1)
        b = np.asarray(in1, np.float32).reshape(p, -1)
        prod = (a * b).astype(np.float32)
        return np.cumsum(prod, axis=1, dtype=np.float32)

    spec = Spec(body=scan(AluOp.ADD, Src0 * Src1), reference=_ref)
    if name not in dvo._SUB_OPCODE_FOR_NAME:
        row = max(dvo._SUB_OPCODE_FOR_NAME.values()) + 1
        assert row < 0x20
        dvo._SUB_OPCODE_FOR_NAME[name] = row
    row = dvo._SUB_OPCODE_FOR_NAME[name]
    shas = {}
    for ver in ("v3", "v4"):
        try:
            uops = lower(spec, ver=ver)
            shas[ver] = DveOpSpec(name=name, opcode=row, uops=uops, rd1_en=True).sha(ver)
        except Exception:
            pass
    op = dvo.DveOp(name, spec, subdim=False, uops_sha=shas)
    if not any(o.name == name for o in dvo.OPS):
        dvo.OPS.append(op)
    dvo.CUSTOM_DVE_SPECS[name] = spec
    _DOT_SCAN = op
    return op


# ----------------------------------------------------------------------------
def _fv(t, base_off_elems, dims):
    """Free-dim view of an SBUF/PSUM tile AP: keep its partition dim, replace
    free dims with explicit [step, count] pairs at an element offset."""
    return bass.AP(tensor=t.tensor, offset=t.offset + base_off_elems,
                   ap=[t.ap[0]] + [list(d) for d in dims])


def build_program():
    """Build the (SPMD-identical) single-core Bass program."""
    if USE_SCAN:
        dot_scan = _get_dot_scan()
    nc = bacc.Bacc("TRN2", target_bir_lowering=False, debug=False)

    # x slice for this core: [ci, n, xp, y]; x pre-padded by 2 on each side
    # (host-zeroed), y contiguous
    xin_d = nc.dram_tensor("xin", [DIN, N_IMG, XP, H], F16, kind="ExternalInput")
    w_d = nc.dram_tensor("w", [KCI, KK * CO], F16, kind="ExternalInput")
    b_d = nc.dram_tensor("b", [1, CO], F32, kind="ExternalInput")
    out_d = nc.dram_tensor("out", [B_LOC, H, Wd, CO], F16, kind="ExternalOutput")

    with tile.TileContext(nc) as tc:
        with (
            tc.tile_pool(name="persist", bufs=1) as persist,
            tc.tile_pool(name="votes", bufs=2) as votes_pool,
            tc.tile_pool(name="small2", bufs=2) as small2,
            tc.tile_pool(name="psum", bufs=2, space="PSUM") as psum_pool,
        ):
            # ---- build the 5x ky-shifted conv input layout on-device.
            # xs[(ky,ci), n, xp, y] = x[n, y+ky-2, xp-2, ci] (zeros outside);
            # the x-pad comes in from the host, so (n, xp) flattens to one
            # stride-32 dim and each per-ky shift DMA is a 3-dim transfer.
            xs = persist.tile([KCI, N_IMG, XP, H], F16, tag="xs")
            nc.vector.memset(xs[:], 0.0)
            for ky in range(KK):
                ylo = max(0, ky - 2)
                yhi = min(H, H + ky - 2)
                dlo = ylo - (ky - 2)
                dhi = yhi - (ky - 2)
                nc.sync.dma_start(
                    out=xs[16 * ky:16 * ky + 16, :, :, dlo:dhi],
                    in_=xin_d.ap()[:, :, :, ylo:yhi],
                )
            wsb = persist.tile([KCI, KK * CO], F16, tag="wsb")
            nc.sync.dma_start(out=wsb[:], in_=w_d.ap())
            bias = persist.tile([128, CO], F32, tag="bias")
            b_ap = b_d.ap()
            nc.sync.dma_start(
                out=bias[:],
                in_=bass.AP(tensor=b_ap.tensor, offset=0, ap=[[0, 128], [1, CO]]),
            )
            ones = persist.tile([128, 1], F32, tag="ones")
            nc.vector.memset(ones[:], 1.0)

            # persistent scratch (DVE-only consumers -> single buffer is fine)
            S = persist.tile([128, 1 + SEG_FREE], F32, tag="S")       # big scan
            S2 = persist.tile([128, 1 + M_STREAM], F32, tag="S2")     # sq scan
            nc.vector.memset(S[:, 0:1], 0.0)
            nc.vector.memset(S2[:, 0:1], 0.0)
            route_d = persist.tile([128, SEG_FREE], F32, tag="route_d")
            preact = persist.tile([128, M_STREAM], F32, tag="preact")
            delta = persist.tile([128, J_STREAM * O], F32, tag="delta")
            den = persist.tile([128, L * O], F32, tag="den")
            rden = persist.tile([128, L * O], F32, tag="rden")
            sqn = persist.tile([128, L * O], F32, tag="sqn")
            tsc = persist.tile([128, L * O], F32, tag="tsc")
            sden = persist.tile([128, J_STREAM], F32, tag="sden")
            srden = persist.tile([128, J_STREAM], F32, tag="srden")

            for b in range(B_LOC):
                for tg in range(N_TG):
                    # ---- conv for this seg --------------------------------
                    votes = votes_pool.tile([128, I, L, CO], F32, tag="votes")
                    for dt in range(L):
                        t = tg * L + dt
                        ps = psum_pool.tile([128, I, CO], F32, tag="ps")
                        for i in range(I):
                            n = b * I + i
                            for kx in range(KK):
                                # stationary = 4 x-cols x 32 y, contiguous 128
                                lhs = _fv(xs,
                                          (n * XP + 4 * t + kx) * H,
                                          [[1, 128]])
                                rhs = _fv(wsb, kx * CO, [[1, CO]])
                                nc.tensor.matmul(
                                    ps[:, i, :],
                                    lhsT=lhs,
                                    rhs=rhs,
                                    start=(kx == 0),
                                    stop=(kx == KK - 1),
                                )
                        # evacuate psum -> votes[:, :, dt, :]
                        nc.scalar.copy(
                            out=_fv(votes, dt * CO, [[L * CO, I], [1, CO]]),
                            in_=ps[:, :, :],
                        )

                    # ---- routing for this seg -----------------------------
                    logits = small2.tile([128, J_STREAM * O], F32, tag="logits")
                    exps = small2.tile([128, J_STREAM * O], F32, tag="exps")
                    route = small2.tile([128, J_STREAM * O], F32, tag="route")
                    n2 = small2.tile([128, L * O], F32, tag="n2")
                    act = small2.tile([128, M_STREAM], F32, tag="act")
                    act16 = small2.tile([128, M_STREAM], F16, tag="act16")

                    # views reused across iterations
                    # votes as stream (m=(dt,od), i): [p][m:512 str1][i:8 str512]
                    v_mi = _fv(votes, 0, [[1, M_STREAM], [M_STREAM, I]])
                    # votes as stream (j=(i,dt), od): [p][j:16 str256][od:256 str1]
                    v_jod = _fv(votes, 0, [[CO, J_STREAM], [1, CO]])

                    for it in range(ROUTINGS):
                        if it > 0:
                            # softmax over o: exps, denom, recip, route
                            nc.scalar.activation(out=exps[:], in_=logits[:],
                                                 func=ACTF.Exp)
                            nc.vector.tensor_reduce(
                                out=sden[:], op=ALU.add, axis=AX.X,
                                in_=_fv(exps, 0, [[O, J_STREAM], [1, O]]))
                            nc.vector.reciprocal(out=srden[:], in_=sden[:])
                            nc.vector.tensor_mul(
                                route[:], exps[:],
                                _fv(srden, 0, [[1, J_STREAM], [0, O]]))
                            # expand route[(i,dt,o)] -> route_d[(dt,od),i]
                            # out element (dt,o,d,i) at dt*2048 + o*128 + d*8 + i
                            nc.scalar.activation(
                                out=_fv(route_d, 0,
                                        [[O * CO // 2, L], [CO // 2, O],
                                         [I, D], [1, I]]),
                                in_=_fv(route, 0, [[O, L], [1, O], [0, D], [O * L, I]]),
                                func=ACTF.Copy)

                        # preact_raw[m] = sum_i route*votes  (fused scan + diff)
                        if USE_SCAN:
                            nc.vector._custom_dve(
                                dot_scan, out=S[:, 1:], in0=v_mi,
                                in1=(_fv(ones, 0, [[0, SEG_FREE]]) if it == 0
                                     else route_d[:]))
                            nc.vector.tensor_sub(
                                preact[:],
                                _fv(S, 1 + (I - 1), [[I, M_STREAM]]),
                                _fv(S, 0, [[I, M_STREAM]]))
                        else:
                            if it == 0:
                                nc.vector.tensor_reduce(
                                    out=preact[:], op=ALU.add, axis=AX.X, in_=v_mi)
                            else:
                                nc.vector.tensor_mul(
                                    _fv(S, 1, [[1, M_STREAM], [M_STREAM, I]]),
                                    v_mi,
                                    _fv(route_d, 0, [[I, M_STREAM], [1, I]]))
                                nc.vector.tensor_reduce(
                                    out=preact[:], op=ALU.add, axis=AX.X,
                                    in_=_fv(S, 1, [[1, M_STREAM], [M_STREAM, I]]))
                        # preact = preact_raw*scale + bias
                        nc.vector.scalar_tensor_tensor(
                            out=preact[:], in0=preact[:],
                            scalar=(1.0 / O) if it == 0 else 1.0,
                            in1=_fv(bias, 0, [[0, L], [1, CO]]),
                            op0=ALU.mult, op1=ALU.add)

                        # squash: n2 = sum_d preact^2 (scan+diff), t = sqrt/(1+n2)
                        if USE_SCAN:
                            nc.vector._custom_dve(
                                dot_scan, out=S2[:, 1:], in0=preact[:],
                                in1=preact[:])
                            nc.vector.tensor_sub(
                                n2[:],
                                _fv(S2, 1 + (D - 1), [[D, L * O]]),
                                _fv(S2, 0, [[D, L * O]]))
                        else:
                            nc.vector.tensor_mul(S2[:, 1:], preact[:], preact[:])
                            nc.vector.tensor_reduce(
                                out=n2[:], op=ALU.add, axis=AX.X,
                                in_=_fv(S2, 1, [[D, L * O], [1, D]]))
                        nc.vector.tensor_scalar_add(den[:], n2[:], 1.0)
                        nc.vector.reciprocal(out=rden[:], in_=den[:])
                        nc.scalar.activation(out=sqn[:], in_=n2[:], func=ACTF.Sqrt)
                        nc.vector.tensor_mul(tsc[:], sqn[:], rden[:])
                        nc.vector.tensor_mul(
                            act[:], preact[:],
                            _fv(tsc, 0, [[1, L * O], [0, D]]))

                        if it < ROUTINGS - 1:
                            # agreement: delta[(i,dt,o)] = sum_d votes*act
                            dtarget = logits if it == 0 else delta
                            if USE_SCAN:
                                nc.vector._custom_dve(
                                    dot_scan, out=S[:, 1:], in0=v_jod,
                                    in1=_fv(act, 0, [[0, I], [1, M_STREAM]]))
                                nc.vector.tensor_sub(
                                    dtarget[:],
                                    _fv(S, 1 + (D - 1), [[D, J_STREAM * O]]),
                                    _fv(S, 0, [[D, J_STREAM * O]]))
                            else:
                                nc.vector.tensor_mul(
                                    _fv(S, 1, [[1, SEG_FREE]]),
                                    v_jod,
                                    _fv(act, 0, [[0, I], [1, M_STREAM]]))
                                nc.vector.tensor_reduce(
                                    out=dtarget[:], op=ALU.add, axis=AX.X,
                                    in_=_fv(S, 1, [[D, J_STREAM * O], [1, D]]))
                            if it > 0:
                                nc.vector.tensor_add(logits[:], logits[:], delta[:])

                    # ---- write act back to HBM (as fp16) ------------------
                    nc.scalar.copy(out=act16[:], in_=act[:])
                    # act16[p=(xx,y), (dt, od)] -> out[b, y, 4*(tg*L+dt)+xx, od]
                    for xx in range(4):
                        dst = bass.AP(
                            tensor=out_d.ap().tensor,
                            offset=(b * H * Wd + 4 * (tg * L) + xx) * CO,
                            ap=[[Wd * CO, 32], [4 * CO, L], [1, CO]],
                        )
                        nc.sync.dma_start(
                            out=dst,
                            in_=act16[32 * xx:32 * xx + 32, :].rearrange(
                                "p (l c) -> p l c", l=L))

    if not nc.is_finalized():
        nc.finalize()
    return nc


# ----------------------------------------------------------------------------
_RUNNER = None


def _build_runner():
    """Compile the program once and build a cached jit(shard_map) callable."""
    from concourse.bass2jax import (
        install_neuronx_cc_hook, _bass_exec_p, partition_id_tensor)

    install_neuronx_cc_hook()
    nc = build_program()

    partition_name = (
        nc.partition_id_tensor.name if nc.partition_id_tensor is not None else None)
    in_names, out_names, out_avals = [], [], []
    for alloc in nc.m.functions[0].allocations:
        if not isinstance(alloc, mybir.MemoryLocationSet):
            continue
        name = alloc.memorylocations[0].name
        if alloc.kind == "ExternalInput":
            if name != partition_name:
                in_names.append(name)
        elif alloc.kind == "ExternalOutput":
            out_names.append(name)
            out_avals.append(jax.core.ShapedArray(
                tuple(alloc.tensor_shape), mybir.dt.np(alloc.dtype)))
    assert in_names == ["xin", "w", "b"], in_names
    assert out_names == ["out"], out_names
    n_params, n_outs = len(in_names), len(out_names)
    names_all = tuple(in_names + out_names
                      + ([partition_name] if partition_name else []))

    extra = {}
    if nc.dbg_addr is not None:
        assert not nc.dbg_callbacks
        extra[nc.dbg_addr.name] = np.zeros((1, 2), np.uint32)
        # dbg tensor rides along as a replicated input; keep things simple by
        # requiring it absent (debug=False above should guarantee this).
        raise RuntimeError("unexpected dbg_addr with debug=False")

    def _body(*args):
        operands = list(args)
        if partition_name is not None:
            operands.append(partition_id_tensor())
        outs = _bass_exec_p.bind(
            *operands,
            out_avals=tuple(out_avals),
            in_names=names_all,
            out_names=tuple(out_names),
            lowering_input_output_aliases=(),
            sim_require_finite=True,
            sim_require_nnan=True,
            nc=nc,
        )
        return tuple(outs)

    devices = jax.devices()[:N_CORES]
    assert len(devices) == N_CORES, f"need {N_CORES} devices, got {len(devices)}"
    mesh = Mesh(np.asarray(devices), ("core",))
    sharded = jax.jit(
        shard_map(_body, mesh=mesh,
                  in_specs=(PartitionSpec("core"),) * (n_params + n_outs),
                  out_specs=(PartitionSpec("core"),) * n_outs,
                  check_rep=False),
        donate_argnums=tuple(range(n_params, n_params + n_outs)),
        keep_unused=True,
    )
    return {"fn": sharded, "out_dev": None}


def _host_prep(x, W, b):
    """Build the global (concatenated-over-cores) device input arrays."""
    x = np.asarray(x, np.float32)
    W = np.asarray(W, np.float32)
    b = np.asarray(b, np.float32)
    # xin_g[(i*16+ci), n, 2+xx, yy] = x[n, yy, xx, i, ci]; x-pad zeroed
    xin_g = np.zeros((N_CORES * DIN, N_IMG, XP, H), np.float16)
    xin_g.reshape(N_CORES, DIN, N_IMG, XP, H)[:, :, :, 2:2 + Wd, :] = \
        x.transpose(3, 4, 0, 2, 1)
    # w2[(ky,ci), (kx,co)]
    w2 = W.transpose(0, 2, 1, 3).reshape(KCI, KK * CO).astype(np.float16)
    w_g = np.tile(w2, (N_CORES, 1))
    b_g = np.tile(np.ascontiguousarray(b.reshape(1, CO)), (N_CORES, 1))
    return xin_g, w_g, b_g


def kernel(x, W, b):
    global _RUNNER
    if _RUNNER is None:
        _RUNNER = _build_runner()
    st = _RUNNER
    xin_g, w_g, b_g = _host_prep(x, W, b)
    out_buf = st["out_dev"]
    if out_buf is None:
        out_buf = np.zeros((N_CORES * B_LOC, H, Wd, CO), np.float16)
    (out_dev,) = st["fn"](xin_g, w_g, b_g, out_buf)
    out_np = np.asarray(out_dev)          # [16, 32, 32, 256] fp16
    st["out_dev"] = out_dev               # donated next call
    # global row 2c+j is core c's routing batch 2c+j -> already batch-ordered
    return out_np.reshape(B_FULL, H, Wd, O, D).astype(np.float32)
